# revision 9
# baseline (speedup 1.0000x reference)
"""Trainium2 Bass kernel for single-query multi-head attention.

Reference computation (B=32, N=4096, D=1024, H=16, dk=dv=64):
    q = (query @ wq).reshape(B, H, dk)          # [B, H, dk]
    k = (key @ wk).reshape(B, N, H, dk)
    v = (value @ wv).reshape(B, N, H, dv)
    scores = einsum("bhd,bnhd->bhn", q, k) / 8
    attn = softmax(scores, axis=-1)
    out = einsum("bhn,bnhd->bhd", attn, v).reshape(B, H*dv)

Key algebraic restructuring (64x FLOP reduction vs naive):
    scores[b,h,n] = key[b,n,:] . R_b[:,h]   where R_b[:,h] = wk[:,h-block] @ q[b,h-block]
    out[b,h-block] = (attn[b,h,:] @ value[b]) @ wv[:,h-block]
so the huge key/value projections ([B,N,D]@[D,D]) are never materialized.

Sharding: data-parallel over batch, 4 batch elements per core, 8 cores,
no collectives. Each core streams its 134MB of key/value once (HBM-bound).
"""

import os
import sys

for _p in ("/opt/trn_rl_repo", os.path.expanduser("~/.axon_site/_ro/trn_rl_repo")):
    if os.path.isdir(_p) and _p not in sys.path:
        sys.path.insert(0, _p)

import numpy as np
from contextlib import ExitStack

from concourse import bass, bacc, mybir, tile, masks
from concourse.bass_utils import run_bass_kernel_spmd

N_CORES = 8
B, N, D = 32, 4096, 1024
H, DK = 16, 64
BL = B // N_CORES          # 4 batch elements per core
NT = 512                   # key/value rows per DMA tile (2 MiB f32 source reads)
NSUB = 128                 # rows per compute subtile (partition dim)
F32 = mybir.dt.float32
BF16 = mybir.dt.bfloat16
EXP = mybir.ActivationFunctionType.Exp


def build_graph(debug=False):
    nc = bacc.Bacc()
    q_ext = nc.declare_dram_parameter("query", [BL, D], F32, isOutput=False)
    k_ext = nc.declare_dram_parameter("key", [BL, N, D], F32, isOutput=False)
    v_ext = nc.declare_dram_parameter("value", [BL, N, D], F32, isOutput=False)
    wq_ext = nc.declare_dram_parameter("wq", [D, D], F32, isOutput=False)
    wk_ext = nc.declare_dram_parameter("wk", [D, D], F32, isOutput=False)
    wv_ext = nc.declare_dram_parameter("wv", [D, D], F32, isOutput=False)
    out_ext = nc.declare_dram_parameter("out", [BL, D], F32, isOutput=True)
    dbg = None
    if debug:
        dbg = {
            "q4": nc.declare_dram_parameter("dbg_q4", [BL, D], F32, isOutput=True),
            "r4t": nc.declare_dram_parameter("dbg_r4t", [BL * H, D], F32, isOutput=True),
            "kt": nc.declare_dram_parameter("dbg_kt", [128, D], F32, isOutput=True),
            "exp": nc.declare_dram_parameter("dbg_exp", [128, H], F32, isOutput=True),
            "shat": nc.declare_dram_parameter("dbg_shat", [H, D], F32, isOutput=True),
            "st0": nc.declare_dram_parameter("dbg_st0", [128, BL * H], F32, isOutput=True),
        }

    with ExitStack() as ctx:
        tc = ctx.enter_context(tile.TileContext(nc))
        _body(ctx, tc, nc, q_ext, k_ext, v_ext, wq_ext, wk_ext, wv_ext, out_ext,
              dbg=dbg)
    return nc


def _body(ctx, tc, nc, q_ext, k_ext, v_ext, wq_ext, wk_ext, wv_ext, out_ext, dbg=None):
    const_pool = ctx.enter_context(tc.tile_pool(name="const", bufs=1))
    wkt_pool = ctx.enter_context(tc.tile_pool(name="wkt", bufs=1))
    r4_pool = ctx.enter_context(tc.tile_pool(name="r4", bufs=1))
    st_pool = ctx.enter_context(tc.tile_pool(name="st", bufs=1))
    wstream = ctx.enter_context(tc.tile_pool(name="wstream", bufs=2))
    key_pool = ctx.enter_context(tc.tile_pool(name="keyld", bufs=5))
    val_pool = ctx.enter_context(tc.tile_pool(name="valld", bufs=5))
    keyt_pool = ctx.enter_context(tc.tile_pool(name="keyt", bufs=4))
    exp_pool = ctx.enter_context(tc.tile_pool(name="expp", bufs=6))
    small_pool = ctx.enter_context(tc.tile_pool(name="small", bufs=1))
    ps_a = ctx.enter_context(tc.tile_pool(name="ps_a", bufs=3, space="PSUM"))
    ps_sc = ctx.enter_context(tc.tile_pool(name="ps_sc", bufs=2, space="PSUM"))
    ps_acc = ctx.enter_context(tc.tile_pool(name="ps_acc", bufs=1, space="PSUM"))
    ps_sum = ctx.enter_context(tc.tile_pool(name="ps_sum", bufs=1, space="PSUM"))

    ident_f = const_pool.tile([128, 128], F32, tag="idf")
    masks.make_identity(nc, ident_f[:])
    ident_b = const_pool.tile([128, 128], BF16, tag="idb")
    masks.make_identity(nc, ident_b[:])
    ones = const_pool.tile([128, 1], BF16, tag="ones")
    nc.vector.memset(ones[:], 1.0)

    # ---------------- prologue: q-projection ----------------
    # query [BL, D] -> qT chunks [128, BL] (contraction dim on partitions)
    q_sb = small_pool.tile([BL, D], F32, tag="q")
    nc.sync.dma_start(q_sb[:], q_ext[:])
    qT = small_pool.tile([128, 8 * BL], F32, tag="qT")
    for jc in range(8):
        pt = ps_a.tile([128, 128], F32, tag="a")
        nc.tensor.transpose(pt[:, :BL], q_sb[:, jc * 128:(jc + 1) * 128],
                            ident_f[:BL, :BL])
        nc.any.tensor_copy(qT[:, jc * BL:(jc + 1) * BL], pt[:, :BL])

    # q4[b, hk] = sum_j query[b, j] * wq[j, hk]   (all 4 batches at once)
    q4_ps = ps_acc.tile([BL, D], F32, tag="acc")
    for jc in range(8):
        wq_t = wstream.tile([128, D], F32, tag="w")
        nc.sync.dma_start(wq_t[:], wq_ext[jc * 128:(jc + 1) * 128, :])
        for half in range(2):
            nc.tensor.matmul(q4_ps[:, half * 512:(half + 1) * 512],
                             qT[:, jc * BL:(jc + 1) * BL],
                             wq_t[:, half * 512:(half + 1) * 512],
                             start=(jc == 0), stop=(jc == 7))
    q4_sb = small_pool.tile([BL, D], F32, tag="q4")
    nc.any.tensor_copy(q4_sb[:], q4_ps[:])
    if dbg:
        nc.sync.dma_start(dbg["q4"][:], q4_sb[:])

    # q4T chunks: [128 hk, BL]
    q4T = small_pool.tile([128, 8 * BL], F32, tag="q4T")
    for hc in range(8):
        pt = ps_a.tile([128, 128], F32, tag="a")
        nc.tensor.transpose(pt[:, :BL], q4_sb[:, hc * 128:(hc + 1) * 128],
                            ident_f[:BL, :BL])
        nc.any.tensor_copy(q4T[:, hc * BL:(hc + 1) * BL], pt[:, :BL])

    # Block-diagonal q: Qbd[hk, b*H + h] = q4[b, hk] iff h == hk // 64
    qbd = []
    for hc in range(8):
        qb = small_pool.tile([128, BL * H], F32, tag=f"qbd{hc}", name=f"qbd{hc}")
        nc.vector.memset(qb[:], 0.0)
        for b in range(BL):
            c0 = b * H + 2 * hc
            nc.vector.tensor_copy(qb[0:64, c0:c0 + 1],
                                  q4T[0:64, hc * BL + b:hc * BL + b + 1])
            nc.vector.tensor_copy(qb[64:128, c0 + 1:c0 + 2],
                                  q4T[64:128, hc * BL + b:hc * BL + b + 1])
        qbd.append(qb)

    # wkT[hk, d] chunks (transpose wk on-chip, once)
    wkT = [wkt_pool.tile([128, D], F32, tag=f"wkt{hc}", name=f"wkt{hc}") for hc in range(8)]
    for dc in range(8):
        wk_t = wstream.tile([128, D], F32, tag="w")
        nc.sync.dma_start(wk_t[:], wk_ext[dc * 128:(dc + 1) * 128, :])
        for hc in range(8):
            pt = ps_a.tile([128, 128], F32, tag="a")
            nc.tensor.transpose(pt[:], wk_t[:, hc * 128:(hc + 1) * 128], ident_f[:])
            nc.any.tensor_copy(wkT[hc][:, dc * 128:(dc + 1) * 128], pt[:])

    # R4T[b*H + h, d] = sum_hk Qbd[hk, b*H+h] * wkT[hk, d]
    r4T_ps = ps_acc.tile([BL * H, D], F32, tag="acc")
    for hc in range(8):
        for half in range(2):
            nc.tensor.matmul(r4T_ps[:, half * 512:(half + 1) * 512],
                             qbd[hc][:], wkT[hc][:, half * 512:(half + 1) * 512],
                             start=(hc == 0), stop=(hc == 7))
    r4T_sb = small_pool.tile([BL * H, D], F32, tag="r4T")
    nc.any.tensor_copy(r4T_sb[:], r4T_ps[:])
    if dbg:
        nc.sync.dma_start(dbg["r4t"][:], r4T_sb[:])

    # R4 chunks [128 d, BL*H] in bf16 (rhs of the scores matmul)
    R4 = []
    for dc in range(8):
        pt = ps_a.tile([128, 128], F32, tag="a")
        nc.tensor.transpose(pt[:, :BL * H], r4T_sb[:, dc * 128:(dc + 1) * 128],
                            ident_f[:BL * H, :BL * H])
        rb = r4_pool.tile([128, BL * H], BF16, tag=f"r4_{dc}", name=f"r4_{dc}")
        nc.any.tensor_copy(rb[:], pt[:, :BL * H])
        R4.append(rb)

    # ---------------- main loop ----------------
    sT = [st_pool.tile([128, BL * H], F32, tag=f"st{dc}", name=f"st{dc}") for dc in range(8)]
    n_tiles = N // NT
    last_sub = N // NSUB - 1
    for b in range(BL):
        kb = k_ext[b].rearrange("(t four p) d -> t p four d", four=4, p=128)
        vb = v_ext[b].rearrange("(t four p) d -> t p four d", four=4, p=128)
        s_ps = ps_acc.tile([H, D], F32, tag="acc")
        sum_ps = ps_sum.tile([H, 1], F32, tag="sum")
        for t in range(n_tiles):
            k_t = key_pool.tile([128, 4 * D], BF16, tag="k")
            nc.gpsimd.dma_start(k_t[:].rearrange("p (four d) -> p four d", four=4),
                                kb[t])
            v_t = val_pool.tile([128, 4 * D], BF16, tag="v")
            nc.gpsimd.dma_start(v_t[:].rearrange("p (four d) -> p four d", four=4),
                                vb[t])
            for s in range(4):
                nt = 4 * t + s
                kT_ps = ps_a.tile([128, D], BF16, tag="a")
                for dc in range(8):
                    nc.tensor.transpose(
                        kT_ps[:, dc * 128:(dc + 1) * 128],
                        k_t[:, s * D + dc * 128:s * D + (dc + 1) * 128],
                        ident_b[:])
                kT_sb = keyt_pool.tile([128, D], BF16, tag="kt")
                nc.vector.tensor_copy(kT_sb[:], kT_ps[:])
                if dbg and b == 0 and nt == 0:
                    kt_f = exp_pool.tile([128, D], F32, tag="ktf", name="ktf")
                    nc.vector.tensor_copy(kt_f[:], kT_sb[:])
                    nc.sync.dma_start(dbg["kt"][:], kt_f[:])
                sc_ps = ps_sc.tile([128, H], F32, tag="sc")
                for dc in range(8):
                    nc.tensor.matmul(sc_ps[:],
                                     kT_sb[:, dc * 128:(dc + 1) * 128],
                                     R4[dc][:, b * H:(b + 1) * H],
                                     start=(dc == 0), stop=(dc == 7))
                # exp(scores / 8); no max-subtraction needed (|scores| < ~3)
                e_sb = exp_pool.tile([128, H], BF16, tag="e")
                nc.scalar.activation(e_sb[:], sc_ps[:], EXP, scale=0.125)
                if dbg and b == 0 and nt == 0:
                    e_f = exp_pool.tile([128, H], F32, tag="ef", name="ef")
                    nc.vector.tensor_copy(e_f[:], e_sb[:])
                    nc.sync.dma_start(dbg["exp"][:], e_f[:])
                first, last = nt == 0, nt == last_sub
                nc.tensor.matmul(s_ps[:, 0:512], e_sb[:],
                                 v_t[:, s * D:s * D + 512],
                                 start=first, stop=last, skip_group_check=True)
                nc.tensor.matmul(s_ps[:, 512:1024], e_sb[:],
                                 v_t[:, s * D + 512:(s + 1) * D],
                                 start=first, stop=last, skip_group_check=True)
                nc.tensor.matmul(sum_ps[:], e_sb[:], ones[:],
                                 start=first, stop=last, skip_group_check=True)
        # batch epilogue: normalize and transpose s
        recip = small_pool.tile([H, 1], F32, tag="recip")
        nc.vector.reciprocal(recip[:], sum_ps[:])
        shat = small_pool.tile([H, D], F32, tag="shat")
        nc.vector.tensor_scalar_mul(shat[:], s_ps[:], recip[:])
        if dbg and b == 0:
            nc.sync.dma_start(dbg["shat"][:], shat[:])
        for dc in range(8):
            pt = ps_a.tile([128, 128], F32, tag="a")
            nc.tensor.transpose(pt[:, :H], shat[:, dc * 128:(dc + 1) * 128],
                                ident_f[:H, :H])
            nc.any.tensor_copy(sT[dc][:, b * H:(b + 1) * H], pt[:, :H])

    # ---------------- output projection ----------------
    # out[b, h*64+j] = sum_d sT[d, b*H+h] * wv[d, h*64+j]
    # One PSUM tile (= one bank) per head: a start=True matmul clears the
    # has_written bits of its WHOLE bank, so interleaved accumulation groups
    # must never share a bank.
    wv_sb = [wstream.tile([128, D], F32, tag=f"wv{dc}", name=f"wv{dc}", bufs=1)
             for dc in range(8)]
    for dc in range(8):
        nc.sync.dma_start(wv_sb[dc][:], wv_ext[dc * 128:(dc + 1) * 128, :])
    if dbg:
        nc.sync.dma_start(dbg["st0"][:], sT[0][:])
    o_sb = small_pool.tile([BL, D], F32, tag="o")
    for h in range(H):
        oh_ps = ps_sc.tile([BL, 64], F32, tag="sc", name=f"oh{h}")
        for dc in range(8):
            nc.tensor.matmul(oh_ps[:],
                             sT[dc][:, h:BL * H:H],
                             wv_sb[dc][:, h * 64:(h + 1) * 64],
                             start=(dc == 0), stop=(dc == 7))
        nc.any.tensor_copy(o_sb[:, h * 64:(h + 1) * 64], oh_ps[:])
    nc.sync.dma_start(out_ext[:], o_sb[:])


_graph_cache = {}


def _get_graph():
    if "nc" not in _graph_cache:
        nc = build_graph()
        # Bacc.finalize runs the sync-wait-splitting passes the TRN2 ISA
        # requires (<=1 wait per instruction); the pjrt path serializes the
        # module as-is, so finalize must happen before run.
        if not nc.is_finalized():
            nc.finalize()
        _graph_cache["nc"] = nc
    return _graph_cache["nc"]


def make_in_maps(query, key, value, wq, wk, wv):
    f = np.float32
    wq, wk, wv = (np.ascontiguousarray(w, dtype=f) for w in (wq, wk, wv))
    maps = []
    for c in range(N_CORES):
        sl = slice(c * BL, (c + 1) * BL)
        maps.append({
            "query": np.ascontiguousarray(query[sl], dtype=f),
            "key": np.ascontiguousarray(key[sl], dtype=f),
            "value": np.ascontiguousarray(value[sl], dtype=f),
            "wq": wq, "wk": wk, "wv": wv,
        })
    return maps


def kernel(query, key, value, wq, wk, wv):
    nc = _get_graph()
    in_maps = make_in_maps(query, key, value, wq, wk, wv)
    res = run_bass_kernel_spmd(nc, in_maps, core_ids=list(range(N_CORES)))
    out = np.concatenate([r["out"] for r in res.results], axis=0)
    return out.astype(np.float32)
